# revision 9
# baseline (speedup 1.0000x reference)
"""Grouped SwiGLU experts (MoE, contiguous per-expert token segments) on 8 trn2 cores.

Strategy: expert-parallel over 512-token work units ("slots").  Host splits the
token rows into per-expert contiguous segments (exactly mirroring the
reference's searchsorted routing), chops each segment into 512-token chunks,
and deals chunks round the 8 cores contiguously so every core runs the same
static program: S slots x [512 tokens x one expert].  Per slot the device
computes  out = (silu(x @ w1) * (x @ w3)) @ w2  with bf16 matmuls (fp32 PSUM
accumulation).  x is pre-transposed on host so no on-device transposes are
needed; h = silu(g1)*g3 is produced directly in [hidden, token] layout which
is exactly the lhsT layout the w2 matmul wants.

kernel(**inputs) -> full [16384, 2048] fp32 output.  Self-contained.
"""

import math
import os

import numpy as np
import ml_dtypes

import concourse.bass as bass
import concourse.tile as tile
from concourse import bacc
from concourse import mybir
from concourse.bass_utils import run_bass_kernel_spmd

N_CORES = 8
D = 2048          # dim_in
H = 1408          # dim_hidden
TOK = 512         # tokens per slot
P = 128           # partitions
D_T = D // P      # 16 d-chunks
H_T = H // P      # 11 hid-chunks
TK = TOK // P     # 4 token tiles per slot

_compiled_cache = {}
last_run_info = {}


def _build_program(S: int, cdt):
    """Per-core SPMD program: S slots, each 512 tokens of one expert."""
    nc = bacc.Bacc()

    xt = nc.declare_dram_parameter("xt", [D, S * TOK], cdt, isOutput=False)
    w1 = nc.declare_dram_parameter("w1", [S, D, H], cdt, isOutput=False)
    w3 = nc.declare_dram_parameter("w3", [S, D, H], cdt, isOutput=False)
    w2 = nc.declare_dram_parameter("w2", [S, H, D], cdt, isOutput=False)
    out = nc.declare_dram_parameter("out", [S * TOK, D], mybir.dt.float32, isOutput=True)

    # hidden-dim split of w1/w3: "lo" = hid chunks [0, H_LO), "hi" = rest.
    # lo tiles are last read at hg == H_LO-1, so next slot's lo prefetch can
    # start mid-phase-1 instead of at phase-1 end.
    H_LO = 6
    LOW = H_LO * P          # 768
    HIW = H - LOW           # 640

    with tile.TileContext(nc) as tc:
        with (
            tc.tile_pool(name="xtp", bufs=2) as xtp,
            tc.tile_pool(name="wp", bufs=1) as wp,
            tc.tile_pool(name="hp", bufs=2) as hp,
            tc.tile_pool(name="w2p", bufs=1) as w2p,
            tc.tile_pool(name="outp", bufs=3) as outp,
            tc.tile_pool(name="tmp", bufs=2) as tmp,
            tc.tile_pool(name="ps", bufs=8, space="PSUM") as psp,
        ):
            for s in range(S):
                # ---- loads for this slot (per-d-chunk tiles, lo then hi) ----
                xt_sb = [None] * D_T
                w1lo = [None] * D_T
                w3lo = [None] * D_T
                w1hi = [None] * D_T
                w3hi = [None] * D_T
                for d in range(D_T):
                    dp = slice(d * P, (d + 1) * P)
                    xt_sb[d] = xtp.tile([P, TOK], cdt, tag=f"xt{d}", bufs=2,
                                        name=f"xt_{s}_{d}")
                    nc.sync.dma_start(out=xt_sb[d][:],
                                      in_=xt[dp, s * TOK:(s + 1) * TOK])
                    w1lo[d] = wp.tile([P, LOW], cdt, tag=f"w1lo{d}",
                                      name=f"w1lo_{s}_{d}")
                    nc.sync.dma_start(out=w1lo[d][:], in_=w1[s, dp, 0:LOW])
                    w3lo[d] = wp.tile([P, LOW], cdt, tag=f"w3lo{d}",
                                      name=f"w3lo_{s}_{d}")
                    nc.sync.dma_start(out=w3lo[d][:], in_=w3[s, dp, 0:LOW])
                for d in range(D_T):
                    dp = slice(d * P, (d + 1) * P)
                    w1hi[d] = wp.tile([P, HIW], cdt, tag=f"w1hi{d}",
                                      name=f"w1hi_{s}_{d}")
                    nc.sync.dma_start(out=w1hi[d][:], in_=w1[s, dp, LOW:H])
                    w3hi[d] = wp.tile([P, HIW], cdt, tag=f"w3hi{d}",
                                      name=f"w3hi_{s}_{d}")
                    nc.sync.dma_start(out=w3hi[d][:], in_=w3[s, dp, LOW:H])
                # w2 loads (per hid-chunk tiles; dh0 half can fully prefetch
                # during phase 1, dh1 reuses the tag's slot after dh0 drains)
                w2_sb = [[None] * H_T for _ in range(2)]
                for dh in range(2):
                    dsl = slice(dh * (D // 2), (dh + 1) * (D // 2))
                    for hc in range(H_T):
                        w2_sb[dh][hc] = w2p.tile([P, D // 2], cdt, tag=f"w2_{hc}",
                                                 name=f"w2sb_{s}_{dh}_{hc}")
                        nc.sync.dma_start(
                            out=w2_sb[dh][hc][:], in_=w2[s, hc * P:(hc + 1) * P, dsl]
                        )

                # ---- phase 1: h[hid, tok] = silu(w1.T x) * (w3.T x) ----
                h_sb = hp.tile([P, H_T, TOK], cdt, tag="h")
                for hg in range(H_T):
                    if hg < H_LO:
                        wa, wb = w1lo, w3lo
                        hsl = slice(hg * P, (hg + 1) * P)
                    else:
                        wa, wb = w1hi, w3hi
                        hsl = slice((hg - H_LO) * P, (hg - H_LO + 1) * P)
                    ps1 = psp.tile([P, TOK], mybir.dt.float32, tag="ps")
                    ps3 = psp.tile([P, TOK], mybir.dt.float32, tag="ps")
                    for d in range(D_T):
                        nc.tensor.matmul(
                            out=ps1[:],
                            lhsT=wa[d][:, hsl],
                            rhs=xt_sb[d][:],
                            start=(d == 0),
                            stop=(d == D_T - 1),
                        )
                    for d in range(D_T):
                        nc.tensor.matmul(
                            out=ps3[:],
                            lhsT=wb[d][:, hsl],
                            rhs=xt_sb[d][:],
                            start=(d == 0),
                            stop=(d == D_T - 1),
                        )
                    sil = tmp.tile([P, TOK], cdt, tag="sil")
                    nc.scalar.activation(
                        out=sil[:], in_=ps1[:], func=mybir.ActivationFunctionType.Silu
                    )
                    nc.vector.tensor_mul(h_sb[:, hg, :], sil[:], ps3[:])

                # ---- phase 2: out[tok, :] = h.T @ w2, dout in two halves ----
                for dh in range(2):
                    dsl = slice(dh * (D // 2), (dh + 1) * (D // 2))
                    pso = [psp.tile([P, TOK], mybir.dt.float32, tag="ps",
                                    name=f"pso_{s}_{dh}_{i}")
                           for i in range(2 * TK)]
                    for hc in range(H_T):
                        for tk in range(TK):
                            lhsT = h_sb[:, hc, tk * P:(tk + 1) * P]
                            for dc in range(2):
                                nc.tensor.matmul(
                                    out=pso[tk * 2 + dc][:],
                                    lhsT=lhsT,
                                    rhs=w2_sb[dh][hc][:, dc * TOK:(dc + 1) * TOK],
                                    start=(hc == 0),
                                    stop=(hc == H_T - 1),
                                )
                    for tk in range(TK):
                        o_sb = outp.tile([P, D // 2], mybir.dt.float32, tag="o")
                        for dc in range(2):
                            nc.vector.tensor_copy(
                                out=o_sb[:, dc * TOK:(dc + 1) * TOK],
                                in_=pso[tk * 2 + dc][:],
                            )
                        nc.gpsimd.dma_start(
                            out=out[s * TOK + tk * P: s * TOK + (tk + 1) * P, dsl],
                            in_=o_sb[:],
                        )
    nc.compile()
    return nc


def _plan(m_sizes, T):
    """Mirror the reference routing: contiguous segments by expert, then chop
    into TOK-sized chunks and deal them contiguously across cores."""
    bounds = np.cumsum(np.asarray(m_sizes, dtype=np.int64))
    E = len(bounds)
    chunks = []  # (expert, row_start, nrows)
    prev = 0
    for e in range(E):
        lo, hi = prev, min(int(bounds[e]), T)
        prev = max(lo, hi)
        seg = hi - lo
        off = lo
        while seg > 0:
            take = min(TOK, seg)
            chunks.append((e, off, take))
            off += take
            seg -= take
    S = max(1, math.ceil(len(chunks) / N_CORES))
    while len(chunks) < N_CORES * S:
        chunks.append((0, 0, 0))  # dummy slot
    per_core = [chunks[c * S:(c + 1) * S] for c in range(N_CORES)]
    return per_core, S


def kernel(x, w1, w2, w3, m_sizes, _trace=False):
    x = np.asarray(x, dtype=np.float32)
    w1 = np.asarray(w1, dtype=np.float32)
    w2 = np.asarray(w2, dtype=np.float32)
    w3 = np.asarray(w3, dtype=np.float32)
    T = x.shape[0]

    per_core, S = _plan(m_sizes, T)

    cdt = mybir.dt.bfloat16
    npdt = ml_dtypes.bfloat16

    key = (S, cdt)
    if key not in _compiled_cache:
        _compiled_cache[key] = _build_program(S, cdt)
    nc = _compiled_cache[key]

    w1b = w1.astype(npdt)
    w2b = w2.astype(npdt)
    w3b = w3.astype(npdt)

    in_maps = []
    for c in range(N_CORES):
        slots = per_core[c]
        seg = np.zeros((S * TOK, D), dtype=np.float32)
        for s, (e, off, ln) in enumerate(slots):
            if ln:
                seg[s * TOK:s * TOK + ln] = x[off:off + ln]
        xt_c = np.ascontiguousarray(seg.T).astype(npdt)
        eids = [e for (e, _, _) in slots]
        in_maps.append({
            "xt": xt_c,
            "w1": np.ascontiguousarray(w1b[eids]),
            "w3": np.ascontiguousarray(w3b[eids]),
            "w2": np.ascontiguousarray(w2b[eids]),
        })

    res = run_bass_kernel_spmd(
        nc, in_maps, list(range(N_CORES)), trace=_trace,
    )

    full = np.zeros((T, D), dtype=np.float32)
    for c in range(N_CORES):
        oc = res.results[c]["out"]
        for s, (e, off, ln) in enumerate(per_core[c]):
            if ln:
                full[off:off + ln] = oc[s * TOK:s * TOK + ln]

    last_run_info.clear()
    last_run_info.update({
        "exec_time_ns": res.exec_time_ns,
        "profile_json": getattr(res, "profile_json", None),
        "S": S,
    })
    return full


# revision 11
# speedup vs baseline: 1.2430x; 1.2430x over previous
"""Grouped SwiGLU experts (MoE, contiguous per-expert token segments) on 8 trn2 cores.

Strategy: expert-parallel over 512-token work units ("slots").  Host splits the
token rows into per-expert contiguous segments (exactly mirroring the
reference's searchsorted routing), chops each segment into 512-token chunks,
and deals chunks round the 8 cores contiguously so every core runs the same
static program: S slots x [512 tokens x one expert].  Per slot the device
computes  out = (silu(x @ w1) * (x @ w3)) @ w2  with bf16 matmuls (fp32 PSUM
accumulation).  x is pre-transposed on host so no on-device transposes are
needed; h = silu(g1)*g3 is produced directly in [hidden, token] layout which
is exactly the lhsT layout the w2 matmul wants.

kernel(**inputs) -> full [16384, 2048] fp32 output.  Self-contained.
"""

import math
import os

import numpy as np
import ml_dtypes

import concourse.bass as bass
import concourse.tile as tile
from concourse import bacc
from concourse import mybir
from concourse.bass_utils import run_bass_kernel_spmd

N_CORES = 8
D = 2048          # dim_in
H = 1408          # dim_hidden
TOK = 512         # tokens per slot
P = 128           # partitions
D_T = D // P      # 16 d-chunks
H_T = H // P      # 11 hid-chunks
TK = TOK // P     # 4 token tiles per slot

_compiled_cache = {}
last_run_info = {}


def _build_program(S: int, cdt):
    """Per-core SPMD program: S slots, each 512 tokens of one expert."""
    nc = bacc.Bacc()

    xt = nc.declare_dram_parameter("xt", [D, S * TOK], cdt, isOutput=False)
    w1 = nc.declare_dram_parameter("w1", [S, D, H], cdt, isOutput=False)
    w3 = nc.declare_dram_parameter("w3", [S, D, H], cdt, isOutput=False)
    w2 = nc.declare_dram_parameter("w2", [S, H, D], cdt, isOutput=False)
    out = nc.declare_dram_parameter("out", [S * TOK, D], mybir.dt.float32, isOutput=True)

    # hidden-dim split of w1/w3: "lo" = hid chunks [0, H_LO), "hi" = rest.
    # lo tiles are last read at hg == H_LO-1, so next slot's lo prefetch can
    # start mid-phase-1 instead of at phase-1 end.
    H_LO = 6
    LOW = H_LO * P          # 768
    HIW = H - LOW           # 640

    with tile.TileContext(nc) as tc:
        with (
            tc.tile_pool(name="xtp", bufs=2) as xtp,
            tc.tile_pool(name="wp", bufs=1) as wp,
            tc.tile_pool(name="hp", bufs=2) as hp,
            tc.tile_pool(name="w2p", bufs=1) as w2p,
            tc.tile_pool(name="outp", bufs=3) as outp,
            tc.tile_pool(name="tmp", bufs=2) as tmp,
            tc.tile_pool(name="ps", bufs=8, space="PSUM") as psp,
        ):
            for s in range(S):
                # ---- loads for this slot (per-d-chunk tiles, lo then hi) ----
                xt_sb = [None] * D_T
                w1lo = [None] * D_T
                w3lo = [None] * D_T
                w1hi = [None] * D_T
                w3hi = [None] * D_T
                for d in range(D_T):
                    dp = slice(d * P, (d + 1) * P)
                    xt_sb[d] = xtp.tile([P, TOK], cdt, tag=f"xt{d}", bufs=2,
                                        name=f"xt_{s}_{d}")
                    nc.sync.dma_start(out=xt_sb[d][:],
                                      in_=xt[dp, s * TOK:(s + 1) * TOK])
                    w1lo[d] = wp.tile([P, LOW], cdt, tag=f"w1lo{d}",
                                      name=f"w1lo_{s}_{d}")
                    nc.sync.dma_start(out=w1lo[d][:], in_=w1[s, dp, 0:LOW])
                    w3lo[d] = wp.tile([P, LOW], cdt, tag=f"w3lo{d}",
                                      name=f"w3lo_{s}_{d}")
                    nc.sync.dma_start(out=w3lo[d][:], in_=w3[s, dp, 0:LOW])
                for d in range(D_T):
                    dp = slice(d * P, (d + 1) * P)
                    w1hi[d] = wp.tile([P, HIW], cdt, tag=f"w1hi{d}",
                                      name=f"w1hi_{s}_{d}")
                    nc.sync.dma_start(out=w1hi[d][:], in_=w1[s, dp, LOW:H])
                    w3hi[d] = wp.tile([P, HIW], cdt, tag=f"w3hi{d}",
                                      name=f"w3hi_{s}_{d}")
                    nc.sync.dma_start(out=w3hi[d][:], in_=w3[s, dp, LOW:H])
                # w2 loads (per hid-chunk tiles; dh0 half can fully prefetch
                # during phase 1, dh1 reuses the tag's slot after dh0 drains)
                w2_sb = [[None] * H_T for _ in range(2)]
                for dh in range(2):
                    dsl = slice(dh * (D // 2), (dh + 1) * (D // 2))
                    for hc in range(H_T):
                        w2_sb[dh][hc] = w2p.tile([P, D // 2], cdt, tag=f"w2_{hc}",
                                                 name=f"w2sb_{s}_{dh}_{hc}")
                        nc.sync.dma_start(
                            out=w2_sb[dh][hc][:], in_=w2[s, hc * P:(hc + 1) * P, dsl]
                        )

                # ---- phase 1: h[hid, tok] = silu(w1.T x) * (w3.T x) ----
                h_sb = hp.tile([P, H_T, TOK], cdt, tag="h")
                for hg in range(H_T):
                    if hg < H_LO:
                        wa, wb = w1lo, w3lo
                        hsl = slice(hg * P, (hg + 1) * P)
                    else:
                        wa, wb = w1hi, w3hi
                        hsl = slice((hg - H_LO) * P, (hg - H_LO + 1) * P)
                    ps1 = psp.tile([P, TOK], mybir.dt.float32, tag="ps")
                    ps3 = psp.tile([P, TOK], mybir.dt.float32, tag="ps")
                    for d in range(D_T):
                        nc.tensor.matmul(
                            out=ps1[:],
                            lhsT=wa[d][:, hsl],
                            rhs=xt_sb[d][:],
                            start=(d == 0),
                            stop=(d == D_T - 1),
                        )
                    for d in range(D_T):
                        nc.tensor.matmul(
                            out=ps3[:],
                            lhsT=wb[d][:, hsl],
                            rhs=xt_sb[d][:],
                            start=(d == 0),
                            stop=(d == D_T - 1),
                        )
                    sil = tmp.tile([P, TOK], cdt, tag="sil")
                    nc.scalar.activation(
                        out=sil[:], in_=ps1[:], func=mybir.ActivationFunctionType.Silu
                    )
                    nc.vector.tensor_mul(h_sb[:, hg, :], sil[:], ps3[:])

                # ---- phase 2: out[tok, :] = h.T @ w2, dout in two halves ----
                for dh in range(2):
                    dsl = slice(dh * (D // 2), (dh + 1) * (D // 2))
                    pso = [psp.tile([P, TOK], mybir.dt.float32, tag="ps",
                                    name=f"pso_{s}_{dh}_{i}")
                           for i in range(2 * TK)]
                    for hc in range(H_T):
                        for tk in range(TK):
                            lhsT = h_sb[:, hc, tk * P:(tk + 1) * P]
                            for dc in range(2):
                                nc.tensor.matmul(
                                    out=pso[tk * 2 + dc][:],
                                    lhsT=lhsT,
                                    rhs=w2_sb[dh][hc][:, dc * TOK:(dc + 1) * TOK],
                                    start=(hc == 0),
                                    stop=(hc == H_T - 1),
                                )
                    for tk in range(TK):
                        o_sb = outp.tile([P, D // 2], mybir.dt.float32, tag="o")
                        for dc in range(2):
                            nc.vector.tensor_copy(
                                out=o_sb[:, dc * TOK:(dc + 1) * TOK],
                                in_=pso[tk * 2 + dc][:],
                            )
                        nc.gpsimd.dma_start(
                            out=out[s * TOK + tk * P: s * TOK + (tk + 1) * P, dsl],
                            in_=o_sb[:],
                        )
    nc.compile()
    return nc


def _plan(m_sizes, T):
    """Mirror the reference routing: contiguous segments by expert, then chop
    into TOK-sized chunks and deal them contiguously across cores."""
    bounds = np.cumsum(np.asarray(m_sizes, dtype=np.int64))
    E = len(bounds)
    chunks = []  # (expert, row_start, nrows)
    prev = 0
    for e in range(E):
        lo, hi = prev, min(int(bounds[e]), T)
        prev = max(lo, hi)
        seg = hi - lo
        off = lo
        while seg > 0:
            take = min(TOK, seg)
            chunks.append((e, off, take))
            off += take
            seg -= take
    S = max(1, math.ceil(len(chunks) / N_CORES))
    while len(chunks) < N_CORES * S:
        chunks.append((0, 0, 0))  # dummy slot
    per_core = [chunks[c * S:(c + 1) * S] for c in range(N_CORES)]
    return per_core, S


def kernel(x, w1, w2, w3, m_sizes, _trace=False):
    x = np.asarray(x, dtype=np.float32)
    w1 = np.asarray(w1, dtype=np.float32)
    w2 = np.asarray(w2, dtype=np.float32)
    w3 = np.asarray(w3, dtype=np.float32)
    T = x.shape[0]
    assert x.shape[1] == D and w1.shape[1:] == (D, H), (x.shape, w1.shape)
    assert w2.shape[1:] == (H, D) and w3.shape[1:] == (D, H), (w2.shape, w3.shape)

    per_core, S = _plan(m_sizes, T)

    cdt = mybir.dt.bfloat16
    npdt = ml_dtypes.bfloat16

    key = (S, cdt)
    if key not in _compiled_cache:
        _compiled_cache[key] = _build_program(S, cdt)
    nc = _compiled_cache[key]

    w1b = w1.astype(npdt)
    w2b = w2.astype(npdt)
    w3b = w3.astype(npdt)

    in_maps = []
    for c in range(N_CORES):
        slots = per_core[c]
        seg = np.zeros((S * TOK, D), dtype=np.float32)
        for s, (e, off, ln) in enumerate(slots):
            if ln:
                seg[s * TOK:s * TOK + ln] = x[off:off + ln]
        xt_c = np.ascontiguousarray(seg.T).astype(npdt)
        eids = [e for (e, _, _) in slots]
        in_maps.append({
            "xt": xt_c,
            "w1": np.ascontiguousarray(w1b[eids]),
            "w3": np.ascontiguousarray(w3b[eids]),
            "w2": np.ascontiguousarray(w2b[eids]),
        })

    try:
        res = run_bass_kernel_spmd(
            nc, in_maps, list(range(N_CORES)), trace=_trace,
        )
    except Exception:
        # transient NRT device errors have been observed once after a fresh
        # compile; a single retry is free if the device truly died
        res = run_bass_kernel_spmd(
            nc, in_maps, list(range(N_CORES)), trace=_trace,
        )

    full = np.zeros((T, D), dtype=np.float32)
    for c in range(N_CORES):
        oc = res.results[c]["out"]
        for s, (e, off, ln) in enumerate(per_core[c]):
            if ln:
                full[off:off + ln] = oc[s * TOK:s * TOK + ln]

    last_run_info.clear()
    last_run_info.update({
        "exec_time_ns": res.exec_time_ns,
        "profile_json": getattr(res, "profile_json", None),
        "S": S,
    })
    return full


# revision 12
# speedup vs baseline: 1.2476x; 1.0037x over previous
"""Grouped SwiGLU experts (MoE, contiguous per-expert token segments) on 8 trn2 cores.

Strategy: expert-parallel over 512-token work units ("slots").  Host splits the
token rows into per-expert contiguous segments (exactly mirroring the
reference's searchsorted routing), chops each segment into 512-token chunks,
and deals chunks round the 8 cores contiguously so every core runs the same
static program: S slots x [512 tokens x one expert].  Per slot the device
computes  out = (silu(x @ w1) * (x @ w3)) @ w2  with bf16 matmuls (fp32 PSUM
accumulation).  x is pre-transposed on host so no on-device transposes are
needed; h = silu(g1)*g3 is produced directly in [hidden, token] layout which
is exactly the lhsT layout the w2 matmul wants.

kernel(**inputs) -> full [16384, 2048] fp32 output.  Self-contained.
"""

import math
import os

import numpy as np
import ml_dtypes

import concourse.bass as bass
import concourse.tile as tile
from concourse import bacc
from concourse import mybir
from concourse.bass_utils import run_bass_kernel_spmd

N_CORES = 8
D = 2048          # dim_in
H = 1408          # dim_hidden
TOK = 512         # tokens per slot
P = 128           # partitions
D_T = D // P      # 16 d-chunks
H_T = H // P      # 11 hid-chunks
TK = TOK // P     # 4 token tiles per slot

_compiled_cache = {}
last_run_info = {}


def _build_program(S: int, cdt):
    """Per-core SPMD program: S slots, each 512 tokens of one expert."""
    nc = bacc.Bacc()

    xt = nc.declare_dram_parameter("xt", [D, S * TOK], cdt, isOutput=False)
    w1 = nc.declare_dram_parameter("w1", [S, D, H], cdt, isOutput=False)
    w3 = nc.declare_dram_parameter("w3", [S, D, H], cdt, isOutput=False)
    w2 = nc.declare_dram_parameter("w2", [S, H, D], cdt, isOutput=False)
    out = nc.declare_dram_parameter("out", [S * TOK, D], mybir.dt.float32, isOutput=True)

    # hidden-dim split of w1/w3: "lo" = hid chunks [0, H_LO), "hi" = rest.
    # lo tiles are last read at hg == H_LO-1, so next slot's lo prefetch can
    # start mid-phase-1 instead of at phase-1 end.
    H_LO = 7
    LOW = H_LO * P
    HIW = H - LOW

    with tile.TileContext(nc) as tc:
        with (
            tc.tile_pool(name="xtp", bufs=2) as xtp,
            tc.tile_pool(name="wp", bufs=1) as wp,
            tc.tile_pool(name="hp", bufs=2) as hp,
            tc.tile_pool(name="w2p", bufs=1) as w2p,
            tc.tile_pool(name="outp", bufs=3) as outp,
            tc.tile_pool(name="tmp", bufs=2) as tmp,
            tc.tile_pool(name="ps", bufs=8, space="PSUM") as psp,
        ):
            for s in range(S):
                # ---- loads for this slot (per-d-chunk tiles, lo then hi) ----
                xt_sb = [None] * D_T
                w1lo = [None] * D_T
                w3lo = [None] * D_T
                w1hi = [None] * D_T
                w3hi = [None] * D_T
                for d in range(D_T):
                    dp = slice(d * P, (d + 1) * P)
                    xt_sb[d] = xtp.tile([P, TOK], cdt, tag=f"xt{d}", bufs=2,
                                        name=f"xt_{s}_{d}")
                    nc.sync.dma_start(out=xt_sb[d][:],
                                      in_=xt[dp, s * TOK:(s + 1) * TOK])
                    w1lo[d] = wp.tile([P, LOW], cdt, tag=f"w1lo{d}",
                                      name=f"w1lo_{s}_{d}")
                    nc.sync.dma_start(out=w1lo[d][:], in_=w1[s, dp, 0:LOW])
                    w3lo[d] = wp.tile([P, LOW], cdt, tag=f"w3lo{d}",
                                      name=f"w3lo_{s}_{d}")
                    nc.sync.dma_start(out=w3lo[d][:], in_=w3[s, dp, 0:LOW])
                for d in range(D_T):
                    dp = slice(d * P, (d + 1) * P)
                    w1hi[d] = wp.tile([P, HIW], cdt, tag=f"w1hi{d}",
                                      name=f"w1hi_{s}_{d}")
                    nc.sync.dma_start(out=w1hi[d][:], in_=w1[s, dp, LOW:H])
                    w3hi[d] = wp.tile([P, HIW], cdt, tag=f"w3hi{d}",
                                      name=f"w3hi_{s}_{d}")
                    nc.sync.dma_start(out=w3hi[d][:], in_=w3[s, dp, LOW:H])
                # w2 loads (per hid-chunk tiles; dh0 half can fully prefetch
                # during phase 1, dh1 reuses the tag's slot after dh0 drains)
                w2_sb = [[None] * H_T for _ in range(2)]
                for dh in range(2):
                    dsl = slice(dh * (D // 2), (dh + 1) * (D // 2))
                    for hc in range(H_T):
                        w2_sb[dh][hc] = w2p.tile([P, D // 2], cdt, tag=f"w2_{hc}",
                                                 name=f"w2sb_{s}_{dh}_{hc}")
                        nc.sync.dma_start(
                            out=w2_sb[dh][hc][:], in_=w2[s, hc * P:(hc + 1) * P, dsl]
                        )

                # ---- phase 1: h[hid, tok] = silu(w1.T x) * (w3.T x) ----
                h_sb = hp.tile([P, H_T, TOK], cdt, tag="h")
                for hg in range(H_T):
                    if hg < H_LO:
                        wa, wb = w1lo, w3lo
                        hsl = slice(hg * P, (hg + 1) * P)
                    else:
                        wa, wb = w1hi, w3hi
                        hsl = slice((hg - H_LO) * P, (hg - H_LO + 1) * P)
                    ps1 = psp.tile([P, TOK], mybir.dt.float32, tag="ps")
                    ps3 = psp.tile([P, TOK], mybir.dt.float32, tag="ps")
                    for d in range(D_T):
                        nc.tensor.matmul(
                            out=ps1[:],
                            lhsT=wa[d][:, hsl],
                            rhs=xt_sb[d][:],
                            start=(d == 0),
                            stop=(d == D_T - 1),
                        )
                    for d in range(D_T):
                        nc.tensor.matmul(
                            out=ps3[:],
                            lhsT=wb[d][:, hsl],
                            rhs=xt_sb[d][:],
                            start=(d == 0),
                            stop=(d == D_T - 1),
                        )
                    sil = tmp.tile([P, TOK], cdt, tag="sil")
                    nc.scalar.activation(
                        out=sil[:], in_=ps1[:], func=mybir.ActivationFunctionType.Silu
                    )
                    nc.vector.tensor_mul(h_sb[:, hg, :], sil[:], ps3[:])

                # ---- phase 2: out[tok, :] = h.T @ w2, dout in two halves ----
                for dh in range(2):
                    dsl = slice(dh * (D // 2), (dh + 1) * (D // 2))
                    pso = [psp.tile([P, TOK], mybir.dt.float32, tag="ps",
                                    name=f"pso_{s}_{dh}_{i}")
                           for i in range(2 * TK)]
                    for hc in range(H_T):
                        for tk in range(TK):
                            lhsT = h_sb[:, hc, tk * P:(tk + 1) * P]
                            for dc in range(2):
                                nc.tensor.matmul(
                                    out=pso[tk * 2 + dc][:],
                                    lhsT=lhsT,
                                    rhs=w2_sb[dh][hc][:, dc * TOK:(dc + 1) * TOK],
                                    start=(hc == 0),
                                    stop=(hc == H_T - 1),
                                )
                    for tk in range(TK):
                        o_sb = outp.tile([P, D // 2], mybir.dt.float32, tag="o")
                        for dc in range(2):
                            nc.vector.tensor_copy(
                                out=o_sb[:, dc * TOK:(dc + 1) * TOK],
                                in_=pso[tk * 2 + dc][:],
                            )
                        nc.gpsimd.dma_start(
                            out=out[s * TOK + tk * P: s * TOK + (tk + 1) * P, dsl],
                            in_=o_sb[:],
                        )
    nc.compile()
    return nc


def _plan(m_sizes, T):
    """Mirror the reference routing: contiguous segments by expert, then chop
    into TOK-sized chunks and deal them contiguously across cores."""
    bounds = np.cumsum(np.asarray(m_sizes, dtype=np.int64))
    E = len(bounds)
    chunks = []  # (expert, row_start, nrows)
    prev = 0
    for e in range(E):
        lo, hi = prev, min(int(bounds[e]), T)
        prev = max(lo, hi)
        seg = hi - lo
        off = lo
        while seg > 0:
            take = min(TOK, seg)
            chunks.append((e, off, take))
            off += take
            seg -= take
    S = max(1, math.ceil(len(chunks) / N_CORES))
    while len(chunks) < N_CORES * S:
        chunks.append((0, 0, 0))  # dummy slot
    per_core = [chunks[c * S:(c + 1) * S] for c in range(N_CORES)]
    return per_core, S


def kernel(x, w1, w2, w3, m_sizes, _trace=False):
    x = np.asarray(x, dtype=np.float32)
    w1 = np.asarray(w1, dtype=np.float32)
    w2 = np.asarray(w2, dtype=np.float32)
    w3 = np.asarray(w3, dtype=np.float32)
    T = x.shape[0]
    assert x.shape[1] == D and w1.shape[1:] == (D, H), (x.shape, w1.shape)
    assert w2.shape[1:] == (H, D) and w3.shape[1:] == (D, H), (w2.shape, w3.shape)

    per_core, S = _plan(m_sizes, T)

    cdt = mybir.dt.bfloat16
    npdt = ml_dtypes.bfloat16

    key = (S, cdt)
    if key not in _compiled_cache:
        _compiled_cache[key] = _build_program(S, cdt)
    nc = _compiled_cache[key]

    w1b = w1.astype(npdt)
    w2b = w2.astype(npdt)
    w3b = w3.astype(npdt)

    in_maps = []
    for c in range(N_CORES):
        slots = per_core[c]
        seg = np.zeros((S * TOK, D), dtype=np.float32)
        for s, (e, off, ln) in enumerate(slots):
            if ln:
                seg[s * TOK:s * TOK + ln] = x[off:off + ln]
        xt_c = np.ascontiguousarray(seg.T).astype(npdt)
        eids = [e for (e, _, _) in slots]
        in_maps.append({
            "xt": xt_c,
            "w1": np.ascontiguousarray(w1b[eids]),
            "w3": np.ascontiguousarray(w3b[eids]),
            "w2": np.ascontiguousarray(w2b[eids]),
        })

    try:
        res = run_bass_kernel_spmd(
            nc, in_maps, list(range(N_CORES)), trace=_trace,
        )
    except Exception:
        # transient NRT device errors have been observed once after a fresh
        # compile; a single retry is free if the device truly died
        res = run_bass_kernel_spmd(
            nc, in_maps, list(range(N_CORES)), trace=_trace,
        )

    full = np.zeros((T, D), dtype=np.float32)
    for c in range(N_CORES):
        oc = res.results[c]["out"]
        for s, (e, off, ln) in enumerate(per_core[c]):
            if ln:
                full[off:off + ln] = oc[s * TOK:s * TOK + ln]

    last_run_info.clear()
    last_run_info.update({
        "exec_time_ns": res.exec_time_ns,
        "profile_json": getattr(res, "profile_json", None),
        "S": S,
    })
    return full
